# revision 1
# baseline (speedup 1.0000x reference)
"""ChunkMHSA (banded local-window attention) Trainium2 kernel.

Full-input contract: kernel(**inputs) takes the complete tensors from
setup_inputs() and returns the full [B, T, D] output.  Internally the
sequence dimension is sharded 8 ways (256 queries per NeuronCore) with a
front/back halo of 6/3 tokens, so each core runs the whole fused pipeline
(LayerNorm -> QKV -> banded softmax(QK^T)V -> output projection ->
residual) independently -- no collectives.

Per-core dataflow (SPMD, one Bass program):
  x[tok,D] f32 --bn_stats--> mean/rstd --ts--> xr f16 --PE transpose-->
  xTr[D,tok] --PE f16 matmuls--> q,k [hk,tok] and vT [tok,hk]
  scores psum[q,s] = mask + q.k ; ACT exp(scale=1/8, accum sums) ;
  normalize on DVE ; PE transpose -> attnT[s,q] ; ctx[hk,q] = vT.T@attnT ;
  out psum[q,D] = ctx.Wo ; ACT evac ; GpSimd residual add ; DMA out.
"""

import os

os.environ.setdefault("JAX_PLATFORMS", "axon")

from contextlib import ExitStack

import numpy as np

import concourse.bass as bass
import concourse.bacc as bacc
import concourse.tile as tile
from concourse import mybir
from concourse.bass_utils import run_bass_kernel_spmd

F32 = mybir.dt.float32
F16 = mybir.dt.float16

B, T, D = 2, 2048, 512
H, DH = 8, 64
WF, WB = 6, 3
LN_EPS = 1e-3
NCORES = 8
TLOC = T // NCORES          # 256 queries per core
TIN = WF + TLOC + WB        # 265 local tokens incl. halo
NTT = 3                     # token tiles per batch (128+128+9)
NQC = 2                     # query chunks of 128 per batch
S = 128 + WF + WB           # 137 keys per query chunk
NEG = -30000.0              # additive mask value (fp16-safe)

_CACHE = {}


def _build_program():
    nc = bacc.Bacc(
        "TRN2", target_bir_lowering=False, debug=False, num_devices=NCORES
    )

    xs = nc.dram_tensor("xs", [B, TIN, D], F32, kind="ExternalInput").ap()
    wall = nc.dram_tensor("wall", [16, 128, D], F16, kind="ExternalInput").ap()
    maskd = nc.dram_tensor("maskd", [NQC, 128, S], F16, kind="ExternalInput").ap()
    eye16d = nc.dram_tensor("eye16", [128, 128], F16, kind="ExternalInput").ap()
    xq32d = nc.dram_tensor("xq32", [B, NQC, 128, D], F32, kind="ExternalInput").ap()
    outd = nc.dram_tensor("out", [B, TLOC, D], F32, kind="ExternalOutput").ap()

    with tile.TileContext(nc) as tc, ExitStack() as ctx:
        _emit(ctx, tc, xs, wall, maskd, eye16d, xq32d, outd)

    nc.compile()
    return nc


def _emit(ctx, tc, xs, wall, maskd, eye16d, xq32d, outd):
    nc = tc.nc
    EXP = mybir.ActivationFunctionType.Exp
    SQRT = mybir.ActivationFunctionType.Sqrt
    COPY = mybir.ActivationFunctionType.Copy
    SUB = mybir.AluOpType.subtract
    MULT = mybir.AluOpType.mult

    consts = ctx.enter_context(tc.tile_pool(name="consts", bufs=1))
    persist = ctx.enter_context(tc.tile_pool(name="persist", bufs=1))
    ln_tmp = ctx.enter_context(tc.tile_pool(name="ln_tmp", bufs=3))
    xr_pool = ctx.enter_context(tc.tile_pool(name="xr", bufs=3))
    attn_tmp = ctx.enter_context(tc.tile_pool(name="attn_tmp", bufs=6))
    # PSUM budget 8 banks: scx(2) proj(2) atm(2) att(1) ctx2(1)
    ps_scx = ctx.enter_context(tc.tile_pool(name="ps_scx", bufs=3, space="PSUM"))
    ps_proj = ctx.enter_context(tc.tile_pool(name="ps_proj", bufs=2, space="PSUM"))
    ps_at = ctx.enter_context(tc.tile_pool(name="ps_at", bufs=1, space="PSUM"))
    ps_att = ctx.enter_context(tc.tile_pool(name="ps_att", bufs=1, space="PSUM"))
    ps_ctx = ctx.enter_context(tc.tile_pool(name="ps_ctx", bufs=1, space="PSUM"))

    # ---- constants / weights (DMA issue spread across idle queues) ----------
    eye16 = consts.tile([128, 128], F16)
    nc.scalar.dma_start(eye16, eye16d)
    xq32 = consts.tile([128, B * NQC, D], F32)
    nc.gpsimd.dma_start(xq32, xq32d.rearrange("b c p d -> p (b c) d"))
    mask_sb = consts.tile([128, NQC, S], F16)
    nc.scalar.dma_start(mask_sb, maskd.rearrange("c p s -> p c s"))
    epst = consts.tile([128, 1], F32)
    nc.vector.memset(epst, LN_EPS)
    # warm the ACT tables (Sqrt/Exp/Copy) during the DMA prologue so the
    # ~1.5us lazy table loads don't land mid-pipeline
    warm = consts.tile([128, 1], F32)
    nc.scalar.activation(out=warm, in_=epst, func=SQRT, bias=epst)
    nc.scalar.activation(out=warm, in_=warm, func=EXP)
    nc.scalar.activation(out=warm, in_=warm, func=COPY)

    # all weights in one DMA: wall[4*widx + j] = chunk j of matrix widx
    w_all = consts.tile([128, 16, D], F16)
    nc.gpsimd.dma_start(w_all, wall.rearrange("m p d -> p m d"))

    def w(name, j):
        widx = "qkvo".index(name)
        return w_all[:, 4 * widx + j, :]

    # ---- x load + LayerNorm + centered/scaled xr + transpose ----------------
    x_sb = persist.tile([128, 2 * NTT, D], F32)
    xtr = persist.tile([128, 4, 2 * 384], F16)   # [dpart, dchunk, b*384+tok]
    q_sb = persist.tile([128, 4, B, TLOC], F16, tag="q_sb")
    k_sb = persist.tile([128, 4, B, TIN], F16, tag="k_sb")
    vt_sb = persist.tile([128, B, NTT, D], F16, tag="vt_sb")
    out_stage = persist.tile([128, B * NQC, D], F32, tag="out_stage")

    for b in range(B):
        nc.gpsimd.memset(x_sb[:, b * NTT + 2, :], 0.0)
    # per-tile x loads so LayerNorm can start on tile 0 early
    for b in range(B):
        eng = nc.sync if b == 0 else nc.scalar
        for i in range(2):
            eng.dma_start(
                x_sb[:, b * NTT + i, :], xs[b, 128 * i : 128 * (i + 1), :]
            )
        eng.dma_start(x_sb[:9, b * NTT + 2, :], xs[b, 256:TIN, :])

    def warm_pe(n, pool, tag, shape):
        fill = pool.tile(shape, F16, tag=tag)
        out = fill[0:9, 0, :].bitcast(F32)
        for _ in range(n):
            nc.tensor.matmul(
                out, eye16[:, 0:9], eye16[:, 0:64], start=True, stop=True
            )

    def emit_ln(b, i):
        xt = x_sb[:, b * NTT + i, :]
        st = ln_tmp.tile([128, 6], F32, tag="st")
        mv = ln_tmp.tile([128, 2], F32, tag="mv")
        nc.vector.bn_stats(out=st, in_=xt)
        nc.vector.bn_aggr(out=mv, in_=st)
        sd = ln_tmp.tile([128, 1], F32, tag="sd")
        nc.scalar.activation(out=sd, in_=mv[:, 1:2], func=SQRT, bias=epst)
        rstd = ln_tmp.tile([128, 1], F32, tag="rstd")
        nc.vector.reciprocal(out=rstd, in_=sd)
        xr = xr_pool.tile([128, D], F16, tag="xr")
        nc.vector.tensor_scalar(
            out=xr, in0=xt, scalar1=mv[:, 0:1], scalar2=rstd,
            op0=SUB, op1=MULT,
        )
        pt = ps_scx.tile([128, 4, 128], F16, tag="scx")
        for j in range(4):
            nc.tensor.transpose(pt[:, j, :], xr[:, 128 * j : 128 * j + 128], eye16)
        nc.scalar.activation(
            out=xtr[:, :, 384 * b + 128 * i : 384 * b + 128 * (i + 1)],
            in_=pt, func=COPY,
        )

    def emit_proj(b):
        # q: queries only (N=256)
        for hkt in range(4):
            pp = ps_proj.tile([128, D], F32, tag="proj")
            for j in range(4):
                nc.tensor.matmul(
                    pp[:, 0:TLOC],
                    w("q", j)[:, 128 * hkt : 128 * (hkt + 1)],
                    xtr[:, j, 384 * b + WF : 384 * b + WF + TLOC],
                    start=(j == 0), stop=(j == 3),
                )
            nc.vector.tensor_copy(q_sb[:, hkt, b, :], pp[:, 0:TLOC])
        # k incl. halo (N=265)
        for hkt in range(4):
            pp = ps_proj.tile([128, D], F32, tag="proj")
            for j in range(4):
                nc.tensor.matmul(
                    pp[:, 0:TIN],
                    w("k", j)[:, 128 * hkt : 128 * (hkt + 1)],
                    xtr[:, j, 384 * b : 384 * b + TIN],
                    start=(j == 0), stop=(j == 3),
                )
            nc.scalar.activation(out=k_sb[:, hkt, b, :], in_=pp[:, 0:TIN], func=COPY)
        # vT per token tile (N=512)
        for i in range(NTT):
            pp = ps_proj.tile([128, D], F32, tag="proj")
            for j in range(4):
                nc.tensor.matmul(
                    pp,
                    xtr[:, j, 384 * b + 128 * i : 384 * b + 128 * (i + 1)],
                    w("v", j),
                    start=(j == 0), stop=(j == 3),
                )
            if i % 2 == 0:
                nc.scalar.activation(out=vt_sb[:, b, i, :], in_=pp, func=COPY)
            else:
                nc.vector.tensor_copy(vt_sb[:, b, i, :], pp)

    def emit_attn(b, cq):
        q0 = 128 * cq
        s0 = 128 * cq
        at_m = ps_at.tile([128, 8, 128], F16, tag="atm")
        at_t = ps_att.tile([9, 8, 128], F16, tag="att")
        ctx2 = ps_ctx.tile([128, 4, 128], F32, tag="ctx2")
        for h in range(8):
            hp = 64 * (h % 2)
            hkt = h // 2
            sc = ps_scx.tile([128, S], F32, tag="scx")
            nc.tensor.matmul(sc, eye16, mask_sb[:, cq, :], start=True, stop=False)
            nc.tensor.matmul(
                sc,
                q_sb[hp : hp + 64, hkt, b, q0 : q0 + 128],
                k_sb[hp : hp + 64, hkt, b, s0 : s0 + S],
                start=False, stop=True,
            )
            ea = attn_tmp.tile([128, S], F16, tag="ea")
            sums = attn_tmp.tile([128, 1], F32, tag="sums")
            nc.scalar.activation(
                out=ea, in_=sc, func=EXP, scale=0.125, accum_out=sums
            )
            rec = attn_tmp.tile([128, 1], F32, tag="rec")
            nc.vector.reciprocal(out=rec, in_=sums)
            ean = attn_tmp.tile([128, S], F16, tag="ean")
            nc.vector.tensor_scalar(
                out=ean, in0=ea, scalar1=rec, scalar2=None, op0=MULT
            )
            nc.tensor.transpose(at_m[:, h, :], ean[:, :128], eye16)
            nc.tensor.transpose(at_t[:, h, :], ean[:, 128:S], eye16)
        atm_sb = attn_tmp.tile([128, 8, 128], F16, tag="atm_sb")
        att_sb = attn_tmp.tile([9, 8, 128], F16, tag="att_sb")
        nc.vector.tensor_copy(atm_sb, at_m)
        nc.vector.tensor_copy(att_sb, at_t)
        warm_pe(12, ps_scx, "scx", [128, 4, 128])
        for h in range(8):
            hp = 64 * (h % 2)
            hkt = h // 2
            nc.tensor.matmul(
                ctx2[hp : hp + 64, hkt, :],
                vt_sb[:, b, cq, 64 * h : 64 * h + 64],
                atm_sb[:, h, :],
                start=True, stop=False,
            )
            nc.tensor.matmul(
                ctx2[hp : hp + 64, hkt, :],
                vt_sb[0:9, b, cq + 1, 64 * h : 64 * h + 64],
                att_sb[0:9, h, :],
                start=False, stop=True,
            )
        ctxn_sb = attn_tmp.tile([128, 4, 128], F16, tag="ctxn_sb")
        nc.scalar.activation(out=ctxn_sb, in_=ctx2, func=COPY)
        op = ps_proj.tile([128, D], F32, tag="proj")
        for j in range(4):
            nc.tensor.matmul(
                op, ctxn_sb[:, j, :], w("o", j),
                start=(j == 0), stop=(j == 3),
            )
        oslot = out_stage[:, b * NQC + cq, :]
        nc.scalar.activation(out=oslot, in_=op, func=COPY)
        if b * NQC + cq < B * NQC - 1:
            nc.gpsimd.tensor_add(oslot, oslot, xq32[:, b * NQC + cq, :])
        else:
            nc.vector.tensor_add(oslot, oslot, xq32[:, b * NQC + cq, :])
        nc.sync.dma_start(outd[b, 128 * cq : 128 * (cq + 1), :], oslot)

    # pipeline: b0 LN -> b0 proj -> (b1 LN) -> b0 attn overlaps b1 proj
    emit_ln(0, 0)
    warm_pe(60, ps_at, "atm", [128, 8, 128])
    for i in range(1, NTT):
        emit_ln(0, i)
    emit_proj(0)
    for i in range(NTT):
        emit_ln(1, i)
    emit_attn(0, 0)
    emit_attn(0, 1)
    emit_proj(1)
    emit_attn(1, 0)
    emit_attn(1, 1)


def _prep_host(inputs):
    """Host-side weight folding and per-core slicing."""
    x = np.asarray(inputs["x"], np.float32)
    gamma = np.asarray(inputs["gamma"], np.float32)
    beta = np.asarray(inputs["beta"], np.float32)
    Wq = np.asarray(inputs["Wq"], np.float32).reshape(D, H * DH)
    Wk = np.asarray(inputs["Wk"], np.float32).reshape(D, H * DH)
    Wv = np.asarray(inputs["Wv"], np.float32).reshape(D, H * DH)
    Wo = np.asarray(inputs["Wo"], np.float32).reshape(H * DH, D)
    bq = np.asarray(inputs["bq"], np.float32).reshape(H * DH)
    bk = np.asarray(inputs["bk"], np.float32).reshape(H * DH)
    bv = np.asarray(inputs["bv"], np.float32).reshape(H * DH)
    bo = np.asarray(inputs["bo"], np.float32).reshape(D)

    Wq2 = gamma[:, None] * Wq
    Wk2 = gamma[:, None] * Wk
    Wv2 = gamma[:, None] * Wv
    cq = bq + beta @ Wq
    ck = bk + beta @ Wk
    cv = bv + beta @ Wv
    if np.any(cq) or np.any(ck):
        raise NotImplementedError("nonzero q/k bias not supported")
    bo_eff = bo + cv @ Wo

    wall = np.concatenate(
        [
            w.reshape(4, 128, H * DH).astype(np.float16)
            for w in (Wq2, Wk2, Wv2)
        ]
        + [Wo.reshape(4, 128, D).astype(np.float16)],
        axis=0,
    )
    wall = np.ascontiguousarray(wall)

    eye16 = np.eye(128, dtype=np.float16)

    in_maps = []
    for c in range(NCORES):
        g0 = TLOC * c - WF
        xs = np.zeros((B, TIN, D), np.float32)
        lo, hi = max(0, g0), min(T, g0 + TIN)
        xs[:, lo - g0 : hi - g0, :] = x[:, lo:hi, :]

        mask = np.full((NQC, 128, S), NEG, np.float16)
        for cqi in range(NQC):
            r = np.arange(128)[:, None]
            sl = np.arange(S)[None, :]
            gj = g0 + 128 * cqi + sl
            valid = (sl - r >= 0) & (sl - r <= WF + WB) & (gj >= 0) & (gj < T)
            mask[cqi][valid] = 0.0

        xq32 = np.ascontiguousarray(
            x[:, TLOC * c : TLOC * (c + 1), :].reshape(B, NQC, 128, D)
        )
        in_maps.append(
            {
                "xs": xs, "wall": wall,
                "maskd": mask, "eye16": eye16, "xq32": xq32,
            }
        )
    return in_maps, bo_eff


def kernel(**inputs) -> np.ndarray:
    if "nc" not in _CACHE:
        _CACHE["nc"] = _build_program()
    nc = _CACHE["nc"]
    in_maps, bo_eff = _prep_host(inputs)
    res = run_bass_kernel_spmd(nc, in_maps, list(range(NCORES)))
    out = np.empty((B, T, D), np.float32)
    for c in range(NCORES):
        out[:, TLOC * c : TLOC * (c + 1), :] = res.results[c]["out"]
    if np.any(bo_eff):
        out += bo_eff
    return out



# revision 8
# speedup vs baseline: 1.0658x; 1.0658x over previous
"""ChunkMHSA (banded local-window attention) Trainium2 kernel.

Full-input contract: kernel(**inputs) takes the complete tensors from
setup_inputs() and returns the full [B, T, D] output.  Internally the
sequence dimension is sharded 8 ways (256 queries per NeuronCore) with a
front/back halo of 6/3 tokens, so each core runs the whole fused pipeline
(LayerNorm -> QKV -> banded softmax(QK^T)V -> output projection ->
residual) independently -- no collectives.

v2 schedule: PE is kept continuously busy (p-state ramp) via an
immediate warm-up burst, all x tiles are DMA'd at t=0 across six queues,
scores for 3 heads share a PSUM bank so exp is 3 batched ACTs per
128-query unit (per-head sums via one DVE tensor_reduce per bank), and
attention units are interleaved with projection chunks so the PE never
waits on the exp->normalize chain.  Residual add reads PSUM directly.
"""

import os

os.environ.setdefault("JAX_PLATFORMS", "axon")

from contextlib import ExitStack

import numpy as np

import concourse.bass as bass
import concourse.bacc as bacc
import concourse.tile as tile
from concourse import mybir
from concourse.bass_utils import run_bass_kernel_spmd

F32 = mybir.dt.float32
F16 = mybir.dt.float16

B, T, D = 2, 2048, 512
H, DH = 8, 64
WF, WB = 6, 3
LN_EPS = 1e-3
NCORES = 8
TLOC = T // NCORES          # 256 queries per core
TIN = WF + TLOC + WB        # 265 local tokens incl. halo
NTT = 3                     # token tiles per batch (128+128+9)
NQC = 2                     # query chunks of 128 per batch
S = 128 + WF + WB           # 137 keys per query chunk
MW = 3 * S                  # 411: three 137-wide head slots per PSUM bank
NEG = -30000.0              # additive mask value (fp16-safe)
WARM_N = 9                  # PE p-state ramp matmuls before real work

_CACHE = {}


def _build_program():
    nc = bacc.Bacc(
        "TRN2", target_bir_lowering=False, debug=False, num_devices=NCORES
    )

    xs = nc.dram_tensor("xs", [B, TIN, D], F32, kind="ExternalInput").ap()
    wall = nc.dram_tensor("wall", [16, 128, D], F16, kind="ExternalInput").ap()
    maskd = nc.dram_tensor("maskd", [NQC, 128, MW], F16, kind="ExternalInput").ap()
    eye16d = nc.dram_tensor("eye16", [128, 128], F16, kind="ExternalInput").ap()
    xq32d = nc.dram_tensor("xq32", [B, NQC, 128, D], F32, kind="ExternalInput").ap()
    outd = nc.dram_tensor("out", [B, TLOC, D], F32, kind="ExternalOutput").ap()

    with tile.TileContext(nc) as tc, ExitStack() as ctx:
        _emit(ctx, tc, xs, wall, maskd, eye16d, xq32d, outd)

    nc.compile()
    return nc


def _emit(ctx, tc, xs, wall, maskd, eye16d, xq32d, outd):
    nc = tc.nc
    EXP = mybir.ActivationFunctionType.Exp
    SQRT = mybir.ActivationFunctionType.Sqrt
    COPY = mybir.ActivationFunctionType.Copy
    SUB = mybir.AluOpType.subtract
    MULT = mybir.AluOpType.mult
    AX = mybir.AxisListType.X

    consts = ctx.enter_context(tc.tile_pool(name="consts", bufs=1))
    persist = ctx.enter_context(tc.tile_pool(name="persist", bufs=1))
    ln_tmp = ctx.enter_context(tc.tile_pool(name="ln_tmp", bufs=3))
    xr_pool = ctx.enter_context(tc.tile_pool(name="xr", bufs=3))
    attn_tmp = ctx.enter_context(tc.tile_pool(name="attn_tmp", bufs=6))
    # PSUM budget 8 banks: sc(3) proj(2) atm(1) att(1) ctx2(1)
    ps_sc = ctx.enter_context(tc.tile_pool(name="ps_sc", bufs=3, space="PSUM"))
    ps_proj = ctx.enter_context(tc.tile_pool(name="ps_proj", bufs=2, space="PSUM"))
    ps_at = ctx.enter_context(tc.tile_pool(name="ps_at", bufs=1, space="PSUM"))
    ps_att = ctx.enter_context(tc.tile_pool(name="ps_att", bufs=1, space="PSUM"))
    ps_ctx = ctx.enter_context(tc.tile_pool(name="ps_ctx", bufs=1, space="PSUM"))

    # ---- persistent SBUF -----------------------------------------------------
    x_sb = persist.tile([128, B * NTT, D], F32)
    xtr = persist.tile([128, 4, B * 384], F16)   # [dpart, dchunk, b*384+tok]
    q_sb = persist.tile([128, 4, B, TLOC], F16, tag="q_sb")
    k_sb = persist.tile([128, 4, B, TIN], F16, tag="k_sb")
    vt_sb = persist.tile([128, B, NTT, D], F16, tag="vt_sb")
    out_stage = persist.tile([128, B * NQC, D], F32, tag="out_stage")
    mus = persist.tile([128, B * NTT], F32, tag="mus")
    rstds = persist.tile([128, B * NTT], F32, tag="rstds")
    recs = persist.tile([128, B * NQC, 8], F32, tag="recs")

    # ---- t=0: memsets + all DMAs spread across idle queues -------------------
    epst = consts.tile([128, 1], F32)
    nc.vector.memset(epst, LN_EPS)
    dummy = consts.tile([128, 256], F16)
    nc.vector.memset(dummy, 0.0)

    # x tiles + weights at t=0, spread across the three DMA-capable queues
    w_all = consts.tile([128, 16, D], F16)
    nc.scalar.dma_start(x_sb[:, 0 * NTT + 1, :], xs[0, 128:256, :])
    nc.scalar.dma_start(
        w_all[:, 0:8, :], wall[0:8].rearrange("m p d -> p m d")
    )
    nc.scalar.dma_start(x_sb[:, 1 * NTT + 1, :], xs[1, 128:256, :])

    nc.gpsimd.memset(x_sb[:, 0 * NTT + 2, :], 0.0)
    nc.gpsimd.memset(x_sb[:, 1 * NTT + 2, :], 0.0)

    nc.sync.dma_start(x_sb[:, 0 * NTT + 0, :], xs[0, 0:128, :])
    nc.sync.dma_start(x_sb[:, 1 * NTT + 0, :], xs[1, 0:128, :])
    nc.sync.dma_start(x_sb[:9, 0 * NTT + 2, :], xs[0, 256:TIN, :])
    eye16 = consts.tile([128, 128], F16)
    nc.sync.dma_start(eye16, eye16d)
    mask_sb = consts.tile([128, NQC, 3, S], F16)
    nc.sync.dma_start(mask_sb, maskd.rearrange("c p s -> p c s"))

    nc.gpsimd.dma_start(x_sb[:9, 1 * NTT + 2, :], xs[1, 256:TIN, :])
    nc.gpsimd.dma_start(
        w_all[:, 8:16, :], wall[8:16].rearrange("m p d -> p m d")
    )
    xq32 = consts.tile([128, B * NQC, D], F32)
    nc.gpsimd.dma_start(xq32, xq32d.rearrange("b c p d -> p (b c) d"))

    def w(name, j):
        widx = "qkvo".index(name)
        return w_all[:, 4 * widx + j, :]

    # SQRT table load now, while everything else is still in DMA
    warm_act = consts.tile([128, 1], F32)
    nc.scalar.activation(out=warm_act, in_=epst, func=SQRT, bias=epst)

    # ---- PE warm-up: ramp the p-state on a zero tile ------------------------
    def warm_pe(n):
        for _ in range(n):
            wt = ps_proj.tile([128, D], F32, tag="proj")
            nc.tensor.matmul(
                wt[:, 0:256], dummy[:, 0:128], dummy, start=True, stop=True
            )

    warm_pe(WARM_N)

    # ---- LayerNorm ----------------------------------------------------------
    def emit_ln_stats(b, i):
        xt = x_sb[:, b * NTT + i, :]
        st = ln_tmp.tile([128, 6], F32, tag="st")
        mv = ln_tmp.tile([128, 2], F32, tag="mv")
        nc.vector.bn_stats(out=st, in_=xt)
        nc.vector.bn_aggr(out=mv, in_=st)
        sd = ln_tmp.tile([128, 1], F32, tag="sd")
        nc.scalar.activation(out=sd, in_=mv[:, 1:2], func=SQRT, bias=epst)
        nc.vector.tensor_copy(mus[:, b * NTT + i : b * NTT + i + 1], mv[:, 0:1])
        return sd

    def emit_ln_xt(b, i, sd, ts_eng):
        xt = x_sb[:, b * NTT + i, :]
        rstd = rstds[:, b * NTT + i : b * NTT + i + 1]
        nc.vector.reciprocal(out=rstd, in_=sd)
        xr = xr_pool.tile([128, D], F16, tag="xr")
        ts_eng.tensor_scalar(
            out=xr, in0=xt,
            scalar1=mus[:, b * NTT + i : b * NTT + i + 1],
            scalar2=rstd,
            op0=SUB, op1=MULT,
        )
        pt = ps_sc.tile([128, 4, 128], F16, tag="sc")
        for j in range(4):
            nc.tensor.transpose(pt[:, j, :], xr[:, 128 * j : 128 * j + 128], eye16)
        nc.scalar.activation(
            out=xtr[:, :, 384 * b + 128 * i : 384 * b + 128 * (i + 1)],
            in_=pt, func=COPY,
        )

    def emit_proj_q(b):
        for hkt in range(4):
            pp = ps_proj.tile([128, D], F32, tag="proj")
            for j in range(4):
                nc.tensor.matmul(
                    pp[:, 0:TLOC],
                    w("q", j)[:, 128 * hkt : 128 * (hkt + 1)],
                    xtr[:, j, 384 * b + WF : 384 * b + WF + TLOC],
                    start=(j == 0), stop=(j == 3),
                )
            nc.vector.tensor_copy(q_sb[:, hkt, b, :], pp[:, 0:TLOC])

    def emit_proj_k(b):
        for hkt in range(4):
            pp = ps_proj.tile([128, D], F32, tag="proj")
            for j in range(4):
                nc.tensor.matmul(
                    pp[:, 0:TIN],
                    w("k", j)[:, 128 * hkt : 128 * (hkt + 1)],
                    xtr[:, j, 384 * b : 384 * b + TIN],
                    start=(j == 0), stop=(j == 3),
                )
            nc.scalar.activation(out=k_sb[:, hkt, b, :], in_=pp[:, 0:TIN], func=COPY)

    def emit_proj_v(b):
        for i in range(NTT):
            pp = ps_proj.tile([128, D], F32, tag="proj")
            for j in range(4):
                nc.tensor.matmul(
                    pp,
                    xtr[:, j, 384 * b + 128 * i : 384 * b + 128 * (i + 1)],
                    w("v", j),
                    start=(j == 0), stop=(j == 3),
                )
            if i % 2 == 0:
                nc.scalar.activation(out=vt_sb[:, b, i, :], in_=pp, func=COPY)
            else:
                nc.vector.tensor_copy(vt_sb[:, b, i, :], pp)

    # ---- attention unit (b, cq): 128 queries, 137 keys, 8 heads -------------
    # scores for heads [3t, 3t+1, 3t+2] share PSUM bank t (137-wide slots)
    HPB = (3, 3, 2)  # heads per bank

    def emit_attn_scores(b, cq, ea_banks):
        q0 = 128 * cq
        s0 = 128 * cq
        for t in range(3):
            nh = HPB[t]
            ea = attn_tmp.tile([128, 3, S], F16, tag=f"ea{t}")
            ea_banks.append(ea)
            for hj in range(nh):
                h = 3 * t + hj
                hp = 64 * (h % 2)
                hkt = h // 2
                sc = ps_sc.tile([128, 3, S], F32, tag="sc")
                nc.tensor.matmul(
                    sc[:, 0, :], eye16, mask_sb[:, cq, 0, :],
                    start=True, stop=False,
                )
                nc.tensor.matmul(
                    sc[:, 0, :],
                    q_sb[hp : hp + 64, hkt, b, q0 : q0 + 128],
                    k_sb[hp : hp + 64, hkt, b, s0 : s0 + S],
                    start=False, stop=True,
                )
                ear = attn_tmp.tile([128, S], F16, tag="ear")
                sums = attn_tmp.tile([128, 1], F32, tag="sums")
                nc.scalar.activation(
                    out=ear, in_=sc[:, 0, :], func=EXP, scale=0.125,
                    accum_out=sums,
                )
                rec = attn_tmp.tile([128, 1], F32, tag="rec")
                nc.vector.reciprocal(out=rec, in_=sums)
                nc.vector.tensor_scalar(
                    out=ea[:, hj, :], in0=ear, scalar1=rec,
                    scalar2=None, op0=MULT,
                )

    def emit_attn_mid(b, cq, ea_banks):
        """transposes + ctx matmuls + evacs for unit (b, cq)."""
        at_m = ps_at.tile([128, 8, 128], F16, tag="atm")
        at_t = ps_att.tile([9, 8, 128], F16, tag="att")
        for h in range(8):
            ea = ea_banks[h // 3]
            hj = h % 3
            nc.tensor.transpose(at_m[:, h, :], ea[:, hj, 0:128], eye16)
            nc.tensor.transpose(at_t[:, h, :], ea[:, hj, 128:S], eye16)
        atm_sb = attn_tmp.tile([128, 8, 128], F16, tag="atm_sb")
        att_sb = attn_tmp.tile([9, 8, 128], F16, tag="att_sb")
        nc.vector.tensor_copy(atm_sb, at_m)
        nc.vector.tensor_copy(att_sb, at_t)
        ctx2 = ps_ctx.tile([128, 4, 128], F32, tag="ctx2")
        for h in range(8):
            hp = 64 * (h % 2)
            hkt = h // 2
            nc.tensor.matmul(
                ctx2[hp : hp + 64, hkt, :],
                vt_sb[:, b, cq, 64 * h : 64 * h + 64],
                atm_sb[:, h, :],
                start=True, stop=False,
            )
            nc.tensor.matmul(
                ctx2[hp : hp + 64, hkt, :],
                vt_sb[0:9, b, cq + 1, 64 * h : 64 * h + 64],
                att_sb[0:9, h, :],
                start=False, stop=True,
            )
        ctxn_sb = attn_tmp.tile([128, 4, 128], F16, tag="ctxn_sb")
        nc.scalar.activation(out=ctxn_sb, in_=ctx2, func=COPY)
        return ctxn_sb

    def emit_attn_out(b, cq, ctxn_sb):
        op = ps_proj.tile([128, D], F32, tag="proj")
        for j in range(4):
            nc.tensor.matmul(
                op, ctxn_sb[:, j, :], w("o", j),
                start=(j == 0), stop=(j == 3),
            )
        oslot = out_stage[:, b * NQC + cq, :]
        nc.vector.tensor_add(oslot, op, xq32[:, b * NQC + cq, :])
        nc.sync.dma_start(outd[b, 128 * cq : 128 * (cq + 1), :], oslot)

    # ---- program order (defines each engine's issue order) ------------------
    sd00 = emit_ln_stats(0, 0)
    sd01 = emit_ln_stats(0, 1)
    # COPY table load early (xtr evac is the first Copy ACT)
    nc.scalar.activation(out=warm_act, in_=warm_act, func=COPY)
    emit_ln_xt(0, 0, sd00, nc.vector)
    sd02 = emit_ln_stats(0, 2)
    emit_ln_xt(0, 1, sd01, nc.vector)
    sd10 = emit_ln_stats(1, 0)
    emit_ln_xt(0, 2, sd02, nc.vector)
    emit_proj_q(0)
    emit_ln_xt(1, 0, sd10, nc.vector)
    sd11 = emit_ln_stats(1, 1)
    emit_proj_k(0)
    emit_ln_xt(1, 1, sd11, nc.vector)
    sd12 = emit_ln_stats(1, 2)
    emit_ln_xt(1, 2, sd12, nc.vector)
    emit_proj_v(0)
    # EXP table load after the last SQRT (2-slot LRU never thrashes mid-run)
    nc.scalar.activation(out=warm_act, in_=warm_act, func=EXP)

    ea00 = []
    emit_attn_scores(0, 0, ea00)
    emit_proj_q(1)
    ctxn00 = emit_attn_mid(0, 0, ea00)
    emit_attn_out(0, 0, ctxn00)

    ea01 = []
    emit_attn_scores(0, 1, ea01)
    emit_proj_k(1)
    ctxn01 = emit_attn_mid(0, 1, ea01)
    emit_attn_out(0, 1, ctxn01)

    ea10 = []
    emit_attn_scores(1, 0, ea10)
    emit_proj_v(1)
    ctxn10 = emit_attn_mid(1, 0, ea10)

    ea11 = []
    emit_attn_scores(1, 1, ea11)
    emit_attn_out(1, 0, ctxn10)
    ctxn11 = emit_attn_mid(1, 1, ea11)
    emit_attn_out(1, 1, ctxn11)


def _prep_host(inputs):
    """Host-side weight folding and per-core slicing."""
    x = np.asarray(inputs["x"], np.float32)
    gamma = np.asarray(inputs["gamma"], np.float32)
    beta = np.asarray(inputs["beta"], np.float32)
    Wq = np.asarray(inputs["Wq"], np.float32).reshape(D, H * DH)
    Wk = np.asarray(inputs["Wk"], np.float32).reshape(D, H * DH)
    Wv = np.asarray(inputs["Wv"], np.float32).reshape(D, H * DH)
    Wo = np.asarray(inputs["Wo"], np.float32).reshape(H * DH, D)
    bq = np.asarray(inputs["bq"], np.float32).reshape(H * DH)
    bk = np.asarray(inputs["bk"], np.float32).reshape(H * DH)
    bv = np.asarray(inputs["bv"], np.float32).reshape(H * DH)
    bo = np.asarray(inputs["bo"], np.float32).reshape(D)

    Wq2 = gamma[:, None] * Wq
    Wk2 = gamma[:, None] * Wk
    Wv2 = gamma[:, None] * Wv
    cq = bq + beta @ Wq
    ck = bk + beta @ Wk
    cv = bv + beta @ Wv
    if np.any(cq) or np.any(ck):
        raise NotImplementedError("nonzero q/k bias not supported")
    bo_eff = bo + cv @ Wo

    wall = np.concatenate(
        [
            w.reshape(4, 128, H * DH).astype(np.float16)
            for w in (Wq2, Wk2, Wv2)
        ]
        + [Wo.reshape(4, 128, D).astype(np.float16)],
        axis=0,
    )
    wall = np.ascontiguousarray(wall)

    eye16 = np.eye(128, dtype=np.float16)

    in_maps = []
    for c in range(NCORES):
        g0 = TLOC * c - WF
        xs = np.zeros((B, TIN, D), np.float32)
        lo, hi = max(0, g0), min(T, g0 + TIN)
        xs[:, lo - g0 : hi - g0, :] = x[:, lo:hi, :]

        mask = np.full((NQC, 128, S), NEG, np.float16)
        for cqi in range(NQC):
            r = np.arange(128)[:, None]
            sl = np.arange(S)[None, :]
            gj = g0 + 128 * cqi + sl
            valid = (sl - r >= 0) & (sl - r <= WF + WB) & (gj >= 0) & (gj < T)
            mask[cqi][valid] = 0.0
        mask3 = np.ascontiguousarray(np.tile(mask, (1, 1, 3)))

        xq32 = np.ascontiguousarray(
            x[:, TLOC * c : TLOC * (c + 1), :].reshape(B, NQC, 128, D)
        )
        in_maps.append(
            {
                "xs": xs, "wall": wall,
                "maskd": mask3, "eye16": eye16, "xq32": xq32,
            }
        )
    return in_maps, bo_eff


def kernel(**inputs) -> np.ndarray:
    if "nc" not in _CACHE:
        _CACHE["nc"] = _build_program()
    nc = _CACHE["nc"]
    in_maps, bo_eff = _prep_host(inputs)
    res = run_bass_kernel_spmd(nc, in_maps, list(range(NCORES)))
    out = np.empty((B, T, D), np.float32)
    for c in range(NCORES):
        out[:, TLOC * c : TLOC * (c + 1), :] = res.results[c]["out"]
    if np.any(bo_eff):
        out += bo_eff
    return out


# revision 10
# speedup vs baseline: 1.1141x; 1.0453x over previous
"""ChunkMHSA (banded local-window attention) Trainium2 kernel.

Full-input contract: kernel(**inputs) takes the complete tensors from
setup_inputs() and returns the full [B, T, D] output.  Internally the
sequence dimension is sharded 8 ways (256 queries per NeuronCore) with a
front/back halo of 6/3 tokens, so each core runs the whole fused pipeline
(LayerNorm -> QKV -> banded softmax(QK^T)V -> output projection ->
residual) independently -- no collectives.

v2 schedule: PE is kept continuously busy (p-state ramp) via an
immediate warm-up burst, all x tiles are DMA'd at t=0 across six queues,
scores for 3 heads share a PSUM bank so exp is 3 batched ACTs per
128-query unit (per-head sums via one DVE tensor_reduce per bank), and
attention units are interleaved with projection chunks so the PE never
waits on the exp->normalize chain.  Residual add reads PSUM directly.
"""

import os

os.environ.setdefault("JAX_PLATFORMS", "axon")

from contextlib import ExitStack

import numpy as np

import concourse.bass as bass
import concourse.bacc as bacc
import concourse.tile as tile
from concourse import mybir
from concourse.bass_utils import run_bass_kernel_spmd

F32 = mybir.dt.float32
F16 = mybir.dt.float16

B, T, D = 2, 2048, 512
H, DH = 8, 64
WF, WB = 6, 3
LN_EPS = 1e-3
NCORES = 8
TLOC = T // NCORES          # 256 queries per core
TIN = WF + TLOC + WB        # 265 local tokens incl. halo
NTT = 3                     # token tiles per batch (128+128+9)
NQC = 2                     # query chunks of 128 per batch
S = 128 + WF + WB           # 137 keys per query chunk
MW = 3 * S                  # 411: three 137-wide head slots per PSUM bank
NEG = -30000.0              # additive mask value (fp16-safe)
WARM_N = 9                  # PE p-state ramp matmuls before real work

_CACHE = {}


def _build_program():
    nc = bacc.Bacc(
        "TRN2", target_bir_lowering=False, debug=False, num_devices=NCORES
    )

    xs = nc.dram_tensor("xs", [B, TIN, D], F32, kind="ExternalInput").ap()
    wall = nc.dram_tensor("wall", [16, 128, D], F16, kind="ExternalInput").ap()
    maskd = nc.dram_tensor("maskd", [NQC, 128, MW], F16, kind="ExternalInput").ap()
    eye16d = nc.dram_tensor("eye16", [128, 128], F16, kind="ExternalInput").ap()
    xq32d = nc.dram_tensor("xq32", [B, NQC, 128, D], F32, kind="ExternalInput").ap()
    outd = nc.dram_tensor("out", [B, TLOC, D], F32, kind="ExternalOutput").ap()

    with tile.TileContext(nc) as tc, ExitStack() as ctx:
        _emit(ctx, tc, xs, wall, maskd, eye16d, xq32d, outd)

    nc.compile()
    return nc


def _emit(ctx, tc, xs, wall, maskd, eye16d, xq32d, outd):
    nc = tc.nc
    EXP = mybir.ActivationFunctionType.Exp
    SQRT = mybir.ActivationFunctionType.Sqrt
    COPY = mybir.ActivationFunctionType.Copy
    SUB = mybir.AluOpType.subtract
    MULT = mybir.AluOpType.mult
    AX = mybir.AxisListType.X

    consts = ctx.enter_context(tc.tile_pool(name="consts", bufs=1))
    persist = ctx.enter_context(tc.tile_pool(name="persist", bufs=1))
    ln_tmp = ctx.enter_context(tc.tile_pool(name="ln_tmp", bufs=3))
    xr_pool = ctx.enter_context(tc.tile_pool(name="xr", bufs=3))
    attn_tmp = ctx.enter_context(tc.tile_pool(name="attn_tmp", bufs=6))
    # PSUM budget 8 banks: sc(3) proj(2) atm(1) att(1) ctx2(1)
    ps_sc = ctx.enter_context(tc.tile_pool(name="ps_sc", bufs=3, space="PSUM"))
    ps_proj = ctx.enter_context(tc.tile_pool(name="ps_proj", bufs=2, space="PSUM"))
    ps_at = ctx.enter_context(tc.tile_pool(name="ps_at", bufs=1, space="PSUM"))
    ps_att = ctx.enter_context(tc.tile_pool(name="ps_att", bufs=1, space="PSUM"))
    ps_ctx = ctx.enter_context(tc.tile_pool(name="ps_ctx", bufs=1, space="PSUM"))

    # ---- persistent SBUF -----------------------------------------------------
    x_sb = persist.tile([128, B * NTT, D], F32)
    xtr = persist.tile([128, 4, B * 384], F16)   # [dpart, dchunk, b*384+tok]
    q_sb = persist.tile([128, 4, B, TLOC], F16, tag="q_sb")
    k_sb = persist.tile([128, 4, B, TIN], F16, tag="k_sb")
    vt_sb = persist.tile([128, B, NTT, D], F16, tag="vt_sb")
    out_stage = persist.tile([128, B * NQC, D], F32, tag="out_stage")
    mus = persist.tile([128, B * NTT], F32, tag="mus")
    rstds = persist.tile([128, B * NTT], F32, tag="rstds")
    recs = persist.tile([128, B * NQC, 8], F32, tag="recs")

    # ---- t=0: memsets + all DMAs spread across idle queues -------------------
    epst = consts.tile([128, 1], F32)
    nc.vector.memset(epst, LN_EPS)
    dummy = consts.tile([128, 256], F16)
    nc.vector.memset(dummy, 0.0)

    # x tiles + weights at t=0, spread across the three DMA-capable queues
    w_all = consts.tile([128, 16, D], F16)
    nc.scalar.dma_start(x_sb[:, 0 * NTT + 1, :], xs[0, 128:256, :])
    nc.scalar.dma_start(
        w_all[:, 0:8, :], wall[0:8].rearrange("m p d -> p m d")
    )
    nc.scalar.dma_start(x_sb[:, 1 * NTT + 1, :], xs[1, 128:256, :])

    nc.gpsimd.memset(x_sb[:, 0 * NTT + 2, :], 0.0)
    nc.gpsimd.memset(x_sb[:, 1 * NTT + 2, :], 0.0)

    nc.sync.dma_start(x_sb[:, 0 * NTT + 0, :], xs[0, 0:128, :])
    nc.sync.dma_start(x_sb[:, 1 * NTT + 0, :], xs[1, 0:128, :])
    nc.sync.dma_start(x_sb[:9, 0 * NTT + 2, :], xs[0, 256:TIN, :])
    eye16 = consts.tile([128, 128], F16)
    nc.sync.dma_start(eye16, eye16d)
    mask_sb = consts.tile([128, NQC, 3, S], F16)
    nc.sync.dma_start(mask_sb, maskd.rearrange("c p s -> p c s"))

    nc.gpsimd.dma_start(x_sb[:9, 1 * NTT + 2, :], xs[1, 256:TIN, :])
    nc.gpsimd.dma_start(
        w_all[:, 8:16, :], wall[8:16].rearrange("m p d -> p m d")
    )
    xq32 = consts.tile([128, B * NQC, D], F32)
    nc.gpsimd.dma_start(xq32, xq32d.rearrange("b c p d -> p (b c) d"))

    def w(name, j):
        widx = "qkvo".index(name)
        return w_all[:, 4 * widx + j, :]

    # SQRT table load now, while everything else is still in DMA
    warm_act = consts.tile([128, 1], F32)
    nc.scalar.activation(out=warm_act, in_=epst, func=SQRT, bias=epst)

    # ---- PE warm-up: ramp the p-state on a zero tile ------------------------
    def warm_pe(n):
        for _ in range(n):
            wt = ps_proj.tile([128, D], F32, tag="proj")
            nc.tensor.matmul(
                wt[:, 0:256], dummy[:, 0:128], dummy, start=True, stop=True
            )

    warm_pe(WARM_N)

    # ---- LayerNorm ----------------------------------------------------------
    def emit_ln_stats(b, i):
        xt = x_sb[:, b * NTT + i, :]
        st = ln_tmp.tile([128, 6], F32, tag="st")
        mv = ln_tmp.tile([128, 2], F32, tag="mv")
        nc.vector.bn_stats(out=st, in_=xt)
        nc.vector.bn_aggr(out=mv, in_=st)
        sd = ln_tmp.tile([128, 1], F32, tag="sd")
        nc.scalar.activation(out=sd, in_=mv[:, 1:2], func=SQRT, bias=epst)
        nc.vector.tensor_copy(mus[:, b * NTT + i : b * NTT + i + 1], mv[:, 0:1])
        return sd

    def emit_ln_xt(b, i, sd, ts_eng):
        xt = x_sb[:, b * NTT + i, :]
        rstd = rstds[:, b * NTT + i : b * NTT + i + 1]
        nc.vector.reciprocal(out=rstd, in_=sd)
        xr = xr_pool.tile([128, D], F16, tag="xr")
        ts_eng.tensor_scalar(
            out=xr, in0=xt,
            scalar1=mus[:, b * NTT + i : b * NTT + i + 1],
            scalar2=rstd,
            op0=SUB, op1=MULT,
        )
        pt = ps_sc.tile([128, 4, 128], F16, tag="sc")
        for j in range(4):
            nc.tensor.transpose(pt[:, j, :], xr[:, 128 * j : 128 * j + 128], eye16)
        nc.scalar.activation(
            out=xtr[:, :, 384 * b + 128 * i : 384 * b + 128 * (i + 1)],
            in_=pt, func=COPY,
        )

    def emit_proj_q(b):
        for hkt in range(4):
            pp = ps_proj.tile([128, D], F32, tag="proj")
            for j in range(4):
                nc.tensor.matmul(
                    pp[:, 0:TLOC],
                    w("q", j)[:, 128 * hkt : 128 * (hkt + 1)],
                    xtr[:, j, 384 * b + WF : 384 * b + WF + TLOC],
                    start=(j == 0), stop=(j == 3),
                )
            nc.vector.tensor_copy(q_sb[:, hkt, b, :], pp[:, 0:TLOC])

    def emit_proj_k(b):
        for hkt in range(4):
            pp = ps_proj.tile([128, D], F32, tag="proj")
            for j in range(4):
                nc.tensor.matmul(
                    pp[:, 0:TIN],
                    w("k", j)[:, 128 * hkt : 128 * (hkt + 1)],
                    xtr[:, j, 384 * b : 384 * b + TIN],
                    start=(j == 0), stop=(j == 3),
                )
            nc.scalar.activation(out=k_sb[:, hkt, b, :], in_=pp[:, 0:TIN], func=COPY)

    def emit_proj_v(b):
        for i in range(NTT):
            pp = ps_proj.tile([128, D], F32, tag="proj")
            for j in range(4):
                nc.tensor.matmul(
                    pp,
                    xtr[:, j, 384 * b + 128 * i : 384 * b + 128 * (i + 1)],
                    w("v", j),
                    start=(j == 0), stop=(j == 3),
                )
            if i % 2 == 0:
                nc.scalar.activation(out=vt_sb[:, b, i, :], in_=pp, func=COPY)
            else:
                nc.vector.tensor_copy(vt_sb[:, b, i, :], pp)

    # ---- attention unit (b, cq): 128 queries, 137 keys, 8 heads -------------
    # scores for heads [3t, 3t+1, 3t+2] share PSUM bank t (137-wide slots)
    HPB = (3, 3, 2)  # heads per bank

    def emit_attn_scores(b, cq, ea_banks):
        q0 = 128 * cq
        s0 = 128 * cq
        rec8 = recs[:, b * NQC + cq, :]
        sums8 = attn_tmp.tile([128, 8], F32, tag="sums")
        for t in range(3):
            nh = HPB[t]
            sc = ps_sc.tile([128, 3, S], F32, tag="sc")
            for hj in range(nh):
                h = 3 * t + hj
                hp = 64 * (h % 2)
                hkt = h // 2
                nc.tensor.matmul(
                    sc[:, hj, :], eye16, mask_sb[:, cq, hj, :],
                    start=True, stop=False,
                )
                nc.tensor.matmul(
                    sc[:, hj, :],
                    q_sb[hp : hp + 64, hkt, b, q0 : q0 + 128],
                    k_sb[hp : hp + 64, hkt, b, s0 : s0 + S],
                    start=False, stop=True,
                )
            ear = attn_tmp.tile([128, 3, S], F16, tag=f"ear{t}")
            nc.scalar.activation(
                out=ear[:, 0:nh, :], in_=sc[:, 0:nh, :], func=EXP, scale=0.125
            )
            nc.vector.tensor_reduce(
                out=sums8[:, 3 * t : 3 * t + nh],
                in_=ear[:, 0:nh, :],
                axis=AX, op=mybir.AluOpType.add,
            )
            nc.vector.reciprocal(
                out=rec8[:, 3 * t : 3 * t + nh], in_=sums8[:, 3 * t : 3 * t + nh]
            )
            ea = attn_tmp.tile([128, 3, S], F16, tag=f"ea{t}")
            ea_banks.append(ea)
            for hj in range(nh):
                h = 3 * t + hj
                nc.vector.tensor_scalar(
                    out=ea[:, hj, :], in0=ear[:, hj, :],
                    scalar1=rec8[:, h : h + 1], scalar2=None, op0=MULT,
                )

    def emit_attn_mid(b, cq, ea_banks):
        """transposes + ctx matmuls + evacs for unit (b, cq)."""
        at_m = ps_at.tile([128, 8, 128], F16, tag="atm")
        at_t = ps_att.tile([9, 8, 128], F16, tag="att")
        for h in range(8):
            ea = ea_banks[h // 3]
            hj = h % 3
            nc.tensor.transpose(at_m[:, h, :], ea[:, hj, 0:128], eye16)
            nc.tensor.transpose(at_t[:, h, :], ea[:, hj, 128:S], eye16)
        atm_sb = attn_tmp.tile([128, 8, 128], F16, tag="atm_sb")
        att_sb = attn_tmp.tile([9, 8, 128], F16, tag="att_sb")
        nc.vector.tensor_copy(atm_sb, at_m)
        nc.vector.tensor_copy(att_sb, at_t)
        ctx2 = ps_ctx.tile([128, 4, 128], F32, tag="ctx2")
        for h in range(8):
            hp = 64 * (h % 2)
            hkt = h // 2
            nc.tensor.matmul(
                ctx2[hp : hp + 64, hkt, :],
                vt_sb[:, b, cq, 64 * h : 64 * h + 64],
                atm_sb[:, h, :],
                start=True, stop=False,
            )
            nc.tensor.matmul(
                ctx2[hp : hp + 64, hkt, :],
                vt_sb[0:9, b, cq + 1, 64 * h : 64 * h + 64],
                att_sb[0:9, h, :],
                start=False, stop=True,
            )
        ctxn_sb = attn_tmp.tile([128, 4, 128], F16, tag="ctxn_sb")
        nc.scalar.activation(out=ctxn_sb, in_=ctx2, func=COPY)
        return ctxn_sb

    def emit_attn_out(b, cq, ctxn_sb):
        op = ps_proj.tile([128, D], F32, tag="proj")
        for j in range(4):
            nc.tensor.matmul(
                op, ctxn_sb[:, j, :], w("o", j),
                start=(j == 0), stop=(j == 3),
            )
        oslot = out_stage[:, b * NQC + cq, :]
        nc.vector.tensor_add(oslot, op, xq32[:, b * NQC + cq, :])
        nc.sync.dma_start(outd[b, 128 * cq : 128 * (cq + 1), :], oslot)

    # ---- program order (defines each engine's issue order) ------------------
    sd00 = emit_ln_stats(0, 0)
    sd01 = emit_ln_stats(0, 1)
    # COPY table load early (xtr evac is the first Copy ACT)
    nc.scalar.activation(out=warm_act, in_=warm_act, func=COPY)
    emit_ln_xt(0, 0, sd00, nc.vector)
    sd02 = emit_ln_stats(0, 2)
    emit_ln_xt(0, 1, sd01, nc.vector)
    sd10 = emit_ln_stats(1, 0)
    emit_ln_xt(0, 2, sd02, nc.vector)
    emit_proj_q(0)
    emit_ln_xt(1, 0, sd10, nc.vector)
    sd11 = emit_ln_stats(1, 1)
    emit_proj_k(0)
    emit_ln_xt(1, 1, sd11, nc.vector)
    sd12 = emit_ln_stats(1, 2)
    emit_ln_xt(1, 2, sd12, nc.vector)
    emit_proj_v(0)
    # EXP table load after the last SQRT (2-slot LRU never thrashes mid-run)
    nc.scalar.activation(out=warm_act, in_=warm_act, func=EXP)

    ea00 = []
    emit_attn_scores(0, 0, ea00)
    emit_proj_q(1)
    ctxn00 = emit_attn_mid(0, 0, ea00)
    emit_attn_out(0, 0, ctxn00)

    ea01 = []
    emit_attn_scores(0, 1, ea01)
    emit_proj_k(1)
    ctxn01 = emit_attn_mid(0, 1, ea01)
    emit_attn_out(0, 1, ctxn01)

    ea10 = []
    emit_attn_scores(1, 0, ea10)
    emit_proj_v(1)
    ctxn10 = emit_attn_mid(1, 0, ea10)

    ea11 = []
    emit_attn_scores(1, 1, ea11)
    emit_attn_out(1, 0, ctxn10)
    ctxn11 = emit_attn_mid(1, 1, ea11)
    emit_attn_out(1, 1, ctxn11)


def _prep_host(inputs):
    """Host-side weight folding and per-core slicing."""
    x = np.asarray(inputs["x"], np.float32)
    gamma = np.asarray(inputs["gamma"], np.float32)
    beta = np.asarray(inputs["beta"], np.float32)
    Wq = np.asarray(inputs["Wq"], np.float32).reshape(D, H * DH)
    Wk = np.asarray(inputs["Wk"], np.float32).reshape(D, H * DH)
    Wv = np.asarray(inputs["Wv"], np.float32).reshape(D, H * DH)
    Wo = np.asarray(inputs["Wo"], np.float32).reshape(H * DH, D)
    bq = np.asarray(inputs["bq"], np.float32).reshape(H * DH)
    bk = np.asarray(inputs["bk"], np.float32).reshape(H * DH)
    bv = np.asarray(inputs["bv"], np.float32).reshape(H * DH)
    bo = np.asarray(inputs["bo"], np.float32).reshape(D)

    Wq2 = gamma[:, None] * Wq
    Wk2 = gamma[:, None] * Wk
    Wv2 = gamma[:, None] * Wv
    cq = bq + beta @ Wq
    ck = bk + beta @ Wk
    cv = bv + beta @ Wv
    if np.any(cq) or np.any(ck):
        raise NotImplementedError("nonzero q/k bias not supported")
    bo_eff = bo + cv @ Wo

    wall = np.concatenate(
        [
            w.reshape(4, 128, H * DH).astype(np.float16)
            for w in (Wq2, Wk2, Wv2)
        ]
        + [Wo.reshape(4, 128, D).astype(np.float16)],
        axis=0,
    )
    wall = np.ascontiguousarray(wall)

    eye16 = np.eye(128, dtype=np.float16)

    in_maps = []
    for c in range(NCORES):
        g0 = TLOC * c - WF
        xs = np.zeros((B, TIN, D), np.float32)
        lo, hi = max(0, g0), min(T, g0 + TIN)
        xs[:, lo - g0 : hi - g0, :] = x[:, lo:hi, :]

        mask = np.full((NQC, 128, S), NEG, np.float16)
        for cqi in range(NQC):
            r = np.arange(128)[:, None]
            sl = np.arange(S)[None, :]
            gj = g0 + 128 * cqi + sl
            valid = (sl - r >= 0) & (sl - r <= WF + WB) & (gj >= 0) & (gj < T)
            mask[cqi][valid] = 0.0
        mask3 = np.ascontiguousarray(np.tile(mask, (1, 1, 3)))

        xq32 = np.ascontiguousarray(
            x[:, TLOC * c : TLOC * (c + 1), :].reshape(B, NQC, 128, D)
        )
        in_maps.append(
            {
                "xs": xs, "wall": wall,
                "maskd": mask3, "eye16": eye16, "xq32": xq32,
            }
        )
    return in_maps, bo_eff


def kernel(**inputs) -> np.ndarray:
    if "nc" not in _CACHE:
        _CACHE["nc"] = _build_program()
    nc = _CACHE["nc"]
    in_maps, bo_eff = _prep_host(inputs)
    res = run_bass_kernel_spmd(nc, in_maps, list(range(NCORES)))
    out = np.empty((B, T, D), np.float32)
    for c in range(NCORES):
        out[:, TLOC * c : TLOC * (c + 1), :] = res.results[c]["out"]
    if np.any(bo_eff):
        out += bo_eff
    return out


# revision 11
# speedup vs baseline: 1.2247x; 1.0993x over previous
"""ChunkMHSA (banded local-window attention) Trainium2 kernel.

Full-input contract: kernel(**inputs) takes the complete tensors from
setup_inputs() and returns the full [B, T, D] output.  Internally the
sequence dimension is sharded 8 ways (256 queries per NeuronCore) with a
front/back halo of 6/3 tokens, so each core runs the whole fused pipeline
(LayerNorm -> QKV -> banded softmax(QK^T)V -> output projection ->
residual) independently -- no collectives.

v2 schedule: PE is kept continuously busy (p-state ramp) via an
immediate warm-up burst, all x tiles are DMA'd at t=0 across six queues,
scores for 3 heads share a PSUM bank so exp is 3 batched ACTs per
128-query unit (per-head sums via one DVE tensor_reduce per bank), and
attention units are interleaved with projection chunks so the PE never
waits on the exp->normalize chain.  Residual add reads PSUM directly.
"""

import os

os.environ.setdefault("JAX_PLATFORMS", "axon")

from contextlib import ExitStack

import numpy as np

import concourse.bass as bass
import concourse.bacc as bacc
import concourse.tile as tile
from concourse import mybir
from concourse.bass_utils import run_bass_kernel_spmd

F32 = mybir.dt.float32
F16 = mybir.dt.float16

B, T, D = 2, 2048, 512
H, DH = 8, 64
WF, WB = 6, 3
LN_EPS = 1e-3
NCORES = 8
TLOC = T // NCORES          # 256 queries per core
TIN = WF + TLOC + WB        # 265 local tokens incl. halo
NTT = 3                     # token tiles per batch (128+128+9)
NQC = 2                     # query chunks of 128 per batch
S = 128 + WF + WB           # 137 keys per query chunk
MW = 3 * S                  # 411: three 137-wide head slots per PSUM bank
NEG = -30000.0              # additive mask value (fp16-safe)
WARM_N = 11                 # PE p-state ramp matmuls before real work

_CACHE = {}


def _build_program():
    nc = bacc.Bacc(
        "TRN2", target_bir_lowering=False, debug=False, num_devices=NCORES
    )

    xs = nc.dram_tensor("xs", [B, TIN, D], F32, kind="ExternalInput").ap()
    wall = nc.dram_tensor("wall", [16, 128, D], F16, kind="ExternalInput").ap()
    maskd = nc.dram_tensor("maskd", [NQC, 128, MW], F16, kind="ExternalInput").ap()
    eye16d = nc.dram_tensor("eye16", [128, 128], F16, kind="ExternalInput").ap()
    xq32d = nc.dram_tensor("xq32", [B, NQC, 128, D], F32, kind="ExternalInput").ap()
    outd = nc.dram_tensor("out", [B, TLOC, D], F32, kind="ExternalOutput").ap()

    with tile.TileContext(nc) as tc, ExitStack() as ctx:
        _emit(ctx, tc, xs, wall, maskd, eye16d, xq32d, outd)

    nc.compile()
    return nc


def _emit(ctx, tc, xs, wall, maskd, eye16d, xq32d, outd):
    nc = tc.nc
    EXP = mybir.ActivationFunctionType.Exp
    SQRT = mybir.ActivationFunctionType.Sqrt
    COPY = mybir.ActivationFunctionType.Copy
    SUB = mybir.AluOpType.subtract
    MULT = mybir.AluOpType.mult
    AX = mybir.AxisListType.X

    consts = ctx.enter_context(tc.tile_pool(name="consts", bufs=1))
    persist = ctx.enter_context(tc.tile_pool(name="persist", bufs=1))
    ln_tmp = ctx.enter_context(tc.tile_pool(name="ln_tmp", bufs=3))
    xr_pool = ctx.enter_context(tc.tile_pool(name="xr", bufs=3))
    attn_tmp = ctx.enter_context(tc.tile_pool(name="attn_tmp", bufs=6))
    # PSUM budget 8 banks: sc(3) proj(2) atm(1) att(1) ctx2(1)
    ps_sc = ctx.enter_context(tc.tile_pool(name="ps_sc", bufs=3, space="PSUM"))
    ps_proj = ctx.enter_context(tc.tile_pool(name="ps_proj", bufs=2, space="PSUM"))
    ps_at = ctx.enter_context(tc.tile_pool(name="ps_at", bufs=1, space="PSUM"))
    ps_att = ctx.enter_context(tc.tile_pool(name="ps_att", bufs=1, space="PSUM"))
    ps_ctx = ctx.enter_context(tc.tile_pool(name="ps_ctx", bufs=1, space="PSUM"))

    # ---- persistent SBUF -----------------------------------------------------
    x_sb = persist.tile([128, B * NTT, D], F32)
    xtr = persist.tile([128, 4, B * 384], F16)   # [dpart, dchunk, b*384+tok]
    q_sb = persist.tile([128, 4, B, TLOC], F16, tag="q_sb")
    k_sb = persist.tile([128, 4, B, TIN], F16, tag="k_sb")
    vt_sb = persist.tile([128, B, NTT, D], F16, tag="vt_sb")
    out_stage = persist.tile([128, B * NQC, D], F32, tag="out_stage")
    mus = persist.tile([128, B * NTT], F32, tag="mus")
    rstds = persist.tile([128, B * NTT], F32, tag="rstds")
    recs = persist.tile([128, B * NQC, 8], F32, tag="recs")

    # ---- t=0: memsets + all DMAs spread across idle queues -------------------
    epst = consts.tile([128, 1], F32)
    nc.vector.memset(epst, LN_EPS)
    dummy = consts.tile([128, 256], F16)
    nc.vector.memset(dummy, 0.0)

    # x tiles + weights at t=0, spread across the three DMA-capable queues
    w_all = consts.tile([128, 16, D], F16)
    nc.scalar.dma_start(x_sb[:, 0 * NTT + 1, :], xs[0, 128:256, :])
    nc.scalar.dma_start(x_sb[:, 1 * NTT + 1, :], xs[1, 128:256, :])
    nc.scalar.dma_start(
        w_all[:, 0:8, :], wall[0:8].rearrange("m p d -> p m d")
    )
    nc.scalar.dma_start(
        w_all[:, 8:16, :], wall[8:16].rearrange("m p d -> p m d")
    )

    nc.gpsimd.memset(x_sb[:, 0 * NTT + 2, :], 0.0)
    nc.gpsimd.memset(x_sb[:, 1 * NTT + 2, :], 0.0)

    nc.sync.dma_start(x_sb[:, 0 * NTT + 0, :], xs[0, 0:128, :])
    nc.sync.dma_start(x_sb[:9, 0 * NTT + 2, :], xs[0, 256:TIN, :])
    nc.sync.dma_start(x_sb[:, 1 * NTT + 0, :], xs[1, 0:128, :])
    eye16 = consts.tile([128, 128], F16)
    nc.sync.dma_start(eye16, eye16d)
    mask_sb = consts.tile([128, NQC, 3, S], F16)
    nc.sync.dma_start(mask_sb, maskd.rearrange("c p s -> p c s"))
    xq32 = consts.tile([128, B * NQC, D], F32)
    nc.sync.dma_start(xq32, xq32d.rearrange("b c p d -> p (b c) d"))

    nc.gpsimd.dma_start(x_sb[:9, 1 * NTT + 2, :], xs[1, 256:TIN, :])

    def w(name, j):
        widx = "qkvo".index(name)
        return w_all[:, 4 * widx + j, :]

    # SQRT table load now, while everything else is still in DMA
    warm_act = consts.tile([128, 1], F32)
    nc.scalar.activation(out=warm_act, in_=epst, func=SQRT, bias=epst)

    # ---- PE warm-up: ramp the p-state on a zero tile ------------------------
    def warm_pe(n):
        for _ in range(n):
            wt = ps_proj.tile([128, D], F32, tag="proj")
            nc.tensor.matmul(
                wt[:, 0:256], dummy[:, 0:128], dummy, start=True, stop=True
            )

    warm_pe(WARM_N)

    # ---- LayerNorm ----------------------------------------------------------
    def emit_ln_stats(b, i):
        xt = x_sb[:, b * NTT + i, :]
        st = ln_tmp.tile([128, 6], F32, tag="st")
        mv = ln_tmp.tile([128, 2], F32, tag="mv")
        nc.vector.bn_stats(out=st, in_=xt)
        nc.vector.bn_aggr(out=mv, in_=st)
        sd = ln_tmp.tile([128, 1], F32, tag="sd")
        nc.scalar.activation(out=sd, in_=mv[:, 1:2], func=SQRT, bias=epst)
        nc.vector.tensor_copy(mus[:, b * NTT + i : b * NTT + i + 1], mv[:, 0:1])
        return sd

    def emit_ln_xt(b, i, sd, ts_eng):
        xt = x_sb[:, b * NTT + i, :]
        rstd = rstds[:, b * NTT + i : b * NTT + i + 1]
        nc.vector.reciprocal(out=rstd, in_=sd)
        xr = xr_pool.tile([128, D], F16, tag="xr")
        ts_eng.tensor_scalar(
            out=xr, in0=xt,
            scalar1=mus[:, b * NTT + i : b * NTT + i + 1],
            scalar2=rstd,
            op0=SUB, op1=MULT,
        )
        pt = ps_sc.tile([128, 4, 128], F16, tag="sc")
        for j in range(4):
            nc.tensor.transpose(pt[:, j, :], xr[:, 128 * j : 128 * j + 128], eye16)
        nc.scalar.activation(
            out=xtr[:, :, 384 * b + 128 * i : 384 * b + 128 * (i + 1)],
            in_=pt, func=COPY,
        )

    def emit_proj_q(b):
        for hkt in range(4):
            pp = ps_proj.tile([128, D], F32, tag="proj")
            for j in range(4):
                nc.tensor.matmul(
                    pp[:, 0:TLOC],
                    w("q", j)[:, 128 * hkt : 128 * (hkt + 1)],
                    xtr[:, j, 384 * b + WF : 384 * b + WF + TLOC],
                    start=(j == 0), stop=(j == 3),
                )
            nc.vector.tensor_copy(q_sb[:, hkt, b, :], pp[:, 0:TLOC])

    def emit_proj_k(b):
        for hkt in range(4):
            pp = ps_proj.tile([128, D], F32, tag="proj")
            for j in range(4):
                nc.tensor.matmul(
                    pp[:, 0:TIN],
                    w("k", j)[:, 128 * hkt : 128 * (hkt + 1)],
                    xtr[:, j, 384 * b : 384 * b + TIN],
                    start=(j == 0), stop=(j == 3),
                )
            nc.scalar.activation(out=k_sb[:, hkt, b, :], in_=pp[:, 0:TIN], func=COPY)

    def emit_proj_v(b):
        for i in range(NTT):
            pp = ps_proj.tile([128, D], F32, tag="proj")
            for j in range(4):
                nc.tensor.matmul(
                    pp,
                    xtr[:, j, 384 * b + 128 * i : 384 * b + 128 * (i + 1)],
                    w("v", j),
                    start=(j == 0), stop=(j == 3),
                )
            if i % 2 == 0:
                nc.scalar.activation(out=vt_sb[:, b, i, :], in_=pp, func=COPY)
            else:
                nc.vector.tensor_copy(vt_sb[:, b, i, :], pp)

    # ---- attention unit (b, cq): 128 queries, 137 keys, 8 heads -------------
    # scores for heads [3t, 3t+1, 3t+2] share PSUM bank t (137-wide slots)
    HPB = (3, 3, 2)  # heads per bank

    def emit_attn_scores(b, cq, ea_banks):
        q0 = 128 * cq
        s0 = 128 * cq
        rec8 = recs[:, b * NQC + cq, :]
        sums8 = attn_tmp.tile([128, 8], F32, tag="sums")
        for t in range(3):
            nh = HPB[t]
            sc = ps_sc.tile([128, 3, S], F32, tag="sc")
            for hj in range(nh):
                h = 3 * t + hj
                hp = 64 * (h % 2)
                hkt = h // 2
                nc.tensor.matmul(
                    sc[:, hj, :], eye16, mask_sb[:, cq, hj, :],
                    start=True, stop=False,
                )
                nc.tensor.matmul(
                    sc[:, hj, :],
                    q_sb[hp : hp + 64, hkt, b, q0 : q0 + 128],
                    k_sb[hp : hp + 64, hkt, b, s0 : s0 + S],
                    start=False, stop=True,
                )
            ear = attn_tmp.tile([128, 3, S], F16, tag=f"ear{t}")
            nc.scalar.activation(
                out=ear[:, 0:nh, :], in_=sc[:, 0:nh, :], func=EXP, scale=0.125
            )
            nc.vector.tensor_reduce(
                out=sums8[:, 3 * t : 3 * t + nh],
                in_=ear[:, 0:nh, :],
                axis=AX, op=mybir.AluOpType.add,
            )
            nc.vector.reciprocal(
                out=rec8[:, 3 * t : 3 * t + nh], in_=sums8[:, 3 * t : 3 * t + nh]
            )
            ea = attn_tmp.tile([128, 3, S], F16, tag=f"ea{t}")
            ea_banks.append(ea)
            for hj in range(nh):
                h = 3 * t + hj
                nc.vector.tensor_scalar(
                    out=ea[:, hj, :], in0=ear[:, hj, :],
                    scalar1=rec8[:, h : h + 1], scalar2=None, op0=MULT,
                )

    def emit_attn_mid(b, cq, ea_banks):
        """transposes + ctx matmuls + evacs for unit (b, cq)."""
        at_m = ps_at.tile([128, 8, 128], F16, tag="atm")
        at_t = ps_att.tile([9, 8, 128], F16, tag="att")
        for h in range(8):
            ea = ea_banks[h // 3]
            hj = h % 3
            nc.tensor.transpose(at_m[:, h, :], ea[:, hj, 0:128], eye16)
            nc.tensor.transpose(at_t[:, h, :], ea[:, hj, 128:S], eye16)
        atm_sb = attn_tmp.tile([128, 8, 128], F16, tag="atm_sb")
        att_sb = attn_tmp.tile([9, 8, 128], F16, tag="att_sb")
        nc.scalar.activation(out=atm_sb, in_=at_m, func=COPY)
        nc.vector.tensor_copy(att_sb, at_t)
        ctx2 = ps_ctx.tile([128, 4, 128], F32, tag="ctx2")
        for h in range(8):
            hp = 64 * (h % 2)
            hkt = h // 2
            nc.tensor.matmul(
                ctx2[hp : hp + 64, hkt, :],
                vt_sb[:, b, cq, 64 * h : 64 * h + 64],
                atm_sb[:, h, :],
                start=True, stop=False,
            )
            nc.tensor.matmul(
                ctx2[hp : hp + 64, hkt, :],
                vt_sb[0:9, b, cq + 1, 64 * h : 64 * h + 64],
                att_sb[0:9, h, :],
                start=False, stop=True,
            )
        ctxn_sb = attn_tmp.tile([128, 4, 128], F16, tag="ctxn_sb")
        nc.scalar.activation(out=ctxn_sb, in_=ctx2, func=COPY)
        return ctxn_sb

    def emit_attn_out(b, cq, ctxn_sb):
        op = ps_proj.tile([128, D], F32, tag="proj")
        for j in range(4):
            nc.tensor.matmul(
                op, ctxn_sb[:, j, :], w("o", j),
                start=(j == 0), stop=(j == 3),
            )
        oslot = out_stage[:, b * NQC + cq, :]
        nc.vector.tensor_add(oslot, op, xq32[:, b * NQC + cq, :])
        nc.sync.dma_start(outd[b, 128 * cq : 128 * (cq + 1), :], oslot)

    # ---- program order (defines each engine's issue order) ------------------
    sd00 = emit_ln_stats(0, 0)
    sd01 = emit_ln_stats(0, 1)
    # COPY table load early (xtr evac is the first Copy ACT)
    nc.scalar.activation(out=warm_act, in_=warm_act, func=COPY)
    emit_ln_xt(0, 0, sd00, nc.vector)
    sd02 = emit_ln_stats(0, 2)
    emit_ln_xt(0, 1, sd01, nc.vector)
    emit_ln_xt(0, 2, sd02, nc.vector)
    sd10 = emit_ln_stats(1, 0)
    emit_proj_q(0)
    emit_ln_xt(1, 0, sd10, nc.vector)
    sd11 = emit_ln_stats(1, 1)
    emit_proj_k(0)
    emit_ln_xt(1, 1, sd11, nc.vector)
    sd12 = emit_ln_stats(1, 2)
    emit_ln_xt(1, 2, sd12, nc.vector)
    emit_proj_v(0)
    # EXP table load after the last SQRT (2-slot LRU never thrashes mid-run)
    nc.scalar.activation(out=warm_act, in_=warm_act, func=EXP)

    ea00 = []
    emit_attn_scores(0, 0, ea00)
    emit_proj_q(1)
    ctxn00 = emit_attn_mid(0, 0, ea00)
    emit_attn_out(0, 0, ctxn00)

    ea01 = []
    emit_attn_scores(0, 1, ea01)
    emit_proj_k(1)
    ctxn01 = emit_attn_mid(0, 1, ea01)
    emit_attn_out(0, 1, ctxn01)

    ea10 = []
    emit_attn_scores(1, 0, ea10)
    emit_proj_v(1)
    ctxn10 = emit_attn_mid(1, 0, ea10)

    ea11 = []
    emit_attn_scores(1, 1, ea11)
    emit_attn_out(1, 0, ctxn10)
    ctxn11 = emit_attn_mid(1, 1, ea11)
    emit_attn_out(1, 1, ctxn11)


def _prep_host(inputs):
    """Host-side weight folding and per-core slicing."""
    x = np.asarray(inputs["x"], np.float32)
    gamma = np.asarray(inputs["gamma"], np.float32)
    beta = np.asarray(inputs["beta"], np.float32)
    Wq = np.asarray(inputs["Wq"], np.float32).reshape(D, H * DH)
    Wk = np.asarray(inputs["Wk"], np.float32).reshape(D, H * DH)
    Wv = np.asarray(inputs["Wv"], np.float32).reshape(D, H * DH)
    Wo = np.asarray(inputs["Wo"], np.float32).reshape(H * DH, D)
    bq = np.asarray(inputs["bq"], np.float32).reshape(H * DH)
    bk = np.asarray(inputs["bk"], np.float32).reshape(H * DH)
    bv = np.asarray(inputs["bv"], np.float32).reshape(H * DH)
    bo = np.asarray(inputs["bo"], np.float32).reshape(D)

    Wq2 = gamma[:, None] * Wq
    Wk2 = gamma[:, None] * Wk
    Wv2 = gamma[:, None] * Wv
    cq = bq + beta @ Wq
    ck = bk + beta @ Wk
    cv = bv + beta @ Wv
    if np.any(cq) or np.any(ck):
        raise NotImplementedError("nonzero q/k bias not supported")
    bo_eff = bo + cv @ Wo

    wall = np.concatenate(
        [
            w.reshape(4, 128, H * DH).astype(np.float16)
            for w in (Wq2, Wk2, Wv2)
        ]
        + [Wo.reshape(4, 128, D).astype(np.float16)],
        axis=0,
    )
    wall = np.ascontiguousarray(wall)

    eye16 = np.eye(128, dtype=np.float16)

    in_maps = []
    for c in range(NCORES):
        g0 = TLOC * c - WF
        xs = np.zeros((B, TIN, D), np.float32)
        lo, hi = max(0, g0), min(T, g0 + TIN)
        xs[:, lo - g0 : hi - g0, :] = x[:, lo:hi, :]

        mask = np.full((NQC, 128, S), NEG, np.float16)
        for cqi in range(NQC):
            r = np.arange(128)[:, None]
            sl = np.arange(S)[None, :]
            gj = g0 + 128 * cqi + sl
            valid = (sl - r >= 0) & (sl - r <= WF + WB) & (gj >= 0) & (gj < T)
            mask[cqi][valid] = 0.0
        mask3 = np.ascontiguousarray(np.tile(mask, (1, 1, 3)))

        xq32 = np.ascontiguousarray(
            x[:, TLOC * c : TLOC * (c + 1), :].reshape(B, NQC, 128, D)
        )
        in_maps.append(
            {
                "xs": xs, "wall": wall,
                "maskd": mask3, "eye16": eye16, "xq32": xq32,
            }
        )
    return in_maps, bo_eff


def kernel(**inputs) -> np.ndarray:
    if "nc" not in _CACHE:
        _CACHE["nc"] = _build_program()
    nc = _CACHE["nc"]
    in_maps, bo_eff = _prep_host(inputs)
    res = run_bass_kernel_spmd(nc, in_maps, list(range(NCORES)))
    out = np.empty((B, T, D), np.float32)
    for c in range(NCORES):
        out[:, TLOC * c : TLOC * (c + 1), :] = res.results[c]["out"]
    if np.any(bo_eff):
        out += bo_eff
    return out
